# revision 24
# baseline (speedup 1.0000x reference)
"""Causal multihead self-attention with RoPE on 8 TRN2 NeuronCores.

Problem: B=2, S=2048, D=1024, H=16 heads, d_k=64, causal, RoPE theta=10000.

Sharding (Megatron-style): core c = 4*b + g handles batch b and the 4 heads
[4g, 4g+4): Wq/Wk/Wv column-parallel, Wo row-parallel; each core emits a
partial [S, D] output and the host sums the 4 partials per batch.

Device kernel (per core), matmul operands bf16 (fp32 PSUM accumulation):
  A) qT/kT = W' @ x.T in a d-on-partition layout; d_k dims permuted on host
     so RoPE is 6 full-width DVE ops per (tensor, qtile); DMA repack to
     per-head-contiguous rows; v packed as [v | ones] per head (ones columns
     produce softmax sums for free in the AV matmul).
  B) Per (qtile, head pair): scoresT [128 kpos, 2, 512 q] (row-group packed,
     2 heads concurrent on PE), exp on ACT, causal mask via DVE 0/1 multiply,
     AV accumulated in a single [128, 2, 512] PSUM pair-tile. Normalization:
     ONE ln + ONE exp(-x) over the whole pair tile (garbage NaNs in the
     unread y-rows), then 2 DVE muls into yT.
  C) partial = yT.T @ Wo via PE, gpsimd copy to SBUF, DMA out.

Scheduling (what makes this faster than the 197us baseline):
  - All phase-critical loads ride ONE HWDGE ring (sync) in exact need order:
    xt0+wq interleaved first => first matmul starts ~1us in (was 19.7us).
    cos/sin loaded as [32, S] once and replicated on-chip (SBUF->SBUF).
  - 12 dummy warmup matmuls during the initial DMA wait keep/ramp the PE
    HAM clock gate so real matmuls run at 2.4 GHz.
  - Phase B's inner loop is ACT(exp)-bound (~970ns/chunk vs ~640ns PE work),
    so phase A of qtile t+1 is INTERLEAVED into phase B of qtile t via a
    budget-paced generator: projection matmuls fill the PE idle gaps that
    exp would otherwise leave (PE queue is in-order, so emission order is
    the schedule).
  - PSUM plan (8 banks): sc 2x[128,2,512] + av_pair 1x[128,2,512] +
    2x[128,512] for interleaved projections; phase C borrows sc/av slots.

Measured on 8 axon-attached TRN2 cores: see test.py output.
"""
import sys

sys.path.insert(0, "/opt/trn_rl_repo")

import numpy as np

import concourse.bacc as bacc
import concourse.hw_specs as hw_specs
import concourse.tile as tile
from concourse import mybir
from concourse.bass_utils import run_bass_kernel_spmd

# Keep Exp and Ln in one activation-table set: hide them from every other set
# so bacc's table-load pass picks natural_log_exp_and_others for both instead
# of thrashing between exp_and_others and natural_log (~2.7us per reload).
_orig_act_tables = hw_specs.get_activation_tables


def _patched_act_tables(arch):
    _E = mybir.ActivationFunctionType.Exp
    _L = mybir.ActivationFunctionType.Ln
    out = {}
    for name, fns in _orig_act_tables(arch).items():
        if name != "natural_log_exp_and_others":
            fns = fns - {_E, _L}
        out[name] = fns
    return out


bacc.get_activation_tables = _patched_act_tables

F32 = mybir.dt.float32
F32R = mybir.dt.float32r
BF16 = mybir.dt.bfloat16
USE_BF16 = True
MM_DT = BF16 if USE_BF16 else F32R
EXP = mybir.ActivationFunctionType.Exp
LN = mybir.ActivationFunctionType.Ln
MUL = mybir.AluOpType.mult
ADD = mybir.AluOpType.add
SUB = mybir.AluOpType.subtract

B, S, D = 2, 2048, 1024
H, DK = 16, 64          # global heads, head dim
HL = 4                  # heads per core
GD = HL * DK            # local width 256
T = S // 512            # 4 q-tiles of 512
C = S // 128            # 16 kpos chunks of 128
DCH = D // 128          # 8 contraction chunks
THETA = 10000.0

_cache = {}


def _build_kernel():
    nc = bacc.Bacc("TRN2", target_bir_lowering=False, debug=False, num_devices=8)

    xT = nc.declare_dram_parameter("xT", [D, S], MM_DT, isOutput=False)
    wq = nc.declare_dram_parameter("wq", [D, GD], MM_DT, isOutput=False)
    wk = nc.declare_dram_parameter("wk", [D, GD], MM_DT, isOutput=False)
    wv = nc.declare_dram_parameter("wv", [D, GD], MM_DT, isOutput=False)
    wo = nc.declare_dram_parameter("wo", [GD, D], MM_DT, isOutput=False)
    ccd = nc.declare_dram_parameter("cc", [32, S], F32, isOutput=False)
    ssd = nc.declare_dram_parameter("ss", [32, S], F32, isOutput=False)
    mskd = nc.declare_dram_parameter("msk", [128, 4, 2, 512], MM_DT,
                                     isOutput=False)
    out = nc.declare_dram_parameter("out", [S, D], F32, isOutput=True)

    with tile.TileContext(nc) as tc:
        with (
            tc.tile_pool(name="consts", bufs=1) as consts,
            tc.tile_pool(name="persist", bufs=1) as persist,
            tc.tile_pool(name="xtp", bufs=4) as xtp,
            tc.tile_pool(name="rtmp", bufs=8) as rtmp,
            tc.tile_pool(name="rop", bufs=3) as rop,
            tc.tile_pool(name="ep", bufs=8) as ep,
            tc.tile_pool(name="rp", bufs=4) as rp,
            tc.tile_pool(name="osb", bufs=4) as osb,
            tc.tile_pool(name="scp", bufs=2, space="PSUM") as scp,
            tc.tile_pool(name="avp", bufs=1, space="PSUM") as avp,
            tc.tile_pool(name="app", bufs=2, space="PSUM") as app,
        ):
            # ---------------- tiles ----------------
            wq_t = consts.tile([128, DCH, GD], MM_DT, tag="wq")
            wk_t = consts.tile([128, DCH, GD], MM_DT, tag="wk")
            wv_t = consts.tile([128, DCH, GD], MM_DT, tag="wv")
            wo_t = consts.tile([128, 2, D], MM_DT, tag="wo")
            cc_t = consts.tile([128, S], F32, tag="cc")
            ss_t = consts.tile([128, S], F32, tag="ss")
            msk_t = consts.tile([128, 4, 2, 512], MM_DT, tag="msk")
            warm_f = consts.tile([128, 128], F32, tag="warmf")
            warm = consts.tile([128, 128], MM_DT, tag="warm")
            gate_scr = consts.tile([128, 1], MM_DT, tag="gscr")

            qT = persist.tile([128, 2, S], MM_DT, tag="qT")
            kT = persist.tile([128, 2, S], MM_DT, tag="kT")
            yT = persist.tile([128, 2, S], MM_DT, tag="yT")
            v_ext = persist.tile([128, C, HL, 2 * DK], MM_DT, tag="vext")

            xts = [xtp.tile([128, DCH, 512], MM_DT, tag="xt", name=f"xt{t}")
                   for t in range(T)]

            # ---------------- DMA schedule ----------------
            # sync (HWDGE #1) ring, strict FIFO = priority order: everything
            # qtile-0-critical, in exact need order.  cc/ss early so the
            # on-chip replication (gpsimd) finishes before the first RoPE.
            xT_v = xT.rearrange("(c p) s -> p c s", p=128)
            wq_v = wq.rearrange("(c p) g -> p c g", p=128)
            nc.sync.dma_start(out=xts[0][:, 0:4, :], in_=xT_v[:, 0:4, 0:512])
            nc.sync.dma_start(out=wq_t[:, 0:4, :], in_=wq_v[:, 0:4, :])
            nc.sync.dma_start(out=cc_t[0:32, :], in_=ccd[:])
            nc.sync.dma_start(out=ss_t[0:32, :], in_=ssd[:])
            nc.sync.dma_start(out=xts[0][:, 4:8, :], in_=xT_v[:, 4:8, 0:512])
            nc.sync.dma_start(out=wq_t[:, 4:8, :], in_=wq_v[:, 4:8, :])
            nc.sync.dma_start(
                out=wk_t[:], in_=wk.rearrange("(c p) g -> p c g", p=128))
            nc.sync.dma_start(
                out=wv_t[:], in_=wv.rearrange("(c p) g -> p c g", p=128))
            nc.sync.dma_start(out=msk_t[:], in_=mskd[:])
            nc.sync.dma_start(out=xts[2][:], in_=xT_v[:, :, 1024:1536])
            # (the scalar-ring xt1 load is emitted from inside A(0), after a
            # gating read of xts[1] bytes, so it only competes for HBM after
            # the critical qtile-0 stream has drained)
            # gpsimd (SWDGE) ring: cos/sin replication to all 4 row quads
            # (dep-gated on the [32, S] loads above).  wo/xt3 are emitted
            # later (after A(0)) behind the qtile-0 repacks.
            for rb in range(1, 4):
                nc.gpsimd.dma_start(out=cc_t[32 * rb:32 * (rb + 1), :],
                                    in_=cc_t[0:32, :])
                nc.gpsimd.dma_start(out=ss_t[32 * rb:32 * (rb + 1), :],
                                    in_=ss_t[0:32, :])

            # ---------------- PE warmup during initial DMA wait ----------
            # (also pre-write the xt1 gate bytes so the gating READ below is
            # of initialized data; the gate read orders the scalar-ring xt1
            # DMA via a tracked WAR dependency)
            nc.vector.memset(xts[1][:, 0:1, 0:1], 0.0)
            nc.vector.memset(warm_f[:], 0.03125)
            nc.vector.tensor_copy(warm[:], warm_f[:])
            wps = app.tile([128, 512], F32, tag="app", name="warmps")
            for i in range(12):
                nc.tensor.matmul(wps[:, 0:128], lhsT=warm[:], rhs=warm[:],
                                 start=(i == 0), stop=(i == 11))

            # v_ext ones halves: even heads cols [64:128], odd heads [0:64].
            # Chunks 0-3 (needed by B(0)) now; the rest after A(0).
            nc.vector.memset(v_ext[:, 0:4, 0::2, DK:2 * DK], 1.0)
            nc.vector.memset(v_ext[:, 0:4, 1::2, 0:DK], 1.0)

            # ---------------- phase A generator ----------------
            # Yields ('pe', est_ns, fn) and ('x', 0, fn) in dependency order.
            def A_gen(t):
                qs = slice(512 * t, 512 * (t + 1))
                xt = xts[t]

                if t == 0:
                    def mk_ps(i):
                        pt = scp.tile([128, 2, 512], F32, tag="sc",
                                      name=f"ps0_{i}")
                        return [pt[:, 0, :], pt[:, 1, :]]

                    pvt = {}

                    def mk_psv(s4):
                        if s4 % 2 == 0:
                            pvt[s4 // 2] = avp.tile([128, 2, 512], F32,
                                                    tag="av",
                                                    name=f"pv0_{s4 // 2}")
                        return pvt[s4 // 2][:, s4 % 2, :]
                else:
                    def mk_ps(i):
                        return [app.tile([128, 512], F32, tag="app",
                                         name=f"ps_{t}_{i}_{oc}")
                                for oc in range(2)]

                    def mk_psv(s4):
                        return app.tile([128, 512], F32, tag="app",
                                        name=f"psv_{t}_{s4}")

                for wi, (w_t, dst) in enumerate(((wq_t, qT), (wk_t, kT))):
                    ccs, sss = cc_t[:, qs], ss_t[:, qs]
                    t1 = rtmp.tile([128, 512], F32, tag="rt")
                    t2 = rtmp.tile([128, 512], F32, tag="rt")
                    t3 = rtmp.tile([128, 512], F32, tag="rt")
                    t4 = rtmp.tile([128, 512], F32, tag="rt")
                    ro = rop.tile([128, 2, 512], MM_DT, tag="ro")
                    pss = mk_ps(wi)
                    for oc in range(2):
                        ps = pss[oc]
                        for d in range(DCH):
                            def mm(ps=ps, w_t=w_t, d=d, oc=oc):
                                nc.tensor.matmul(
                                    ps,
                                    lhsT=w_t[:, d, 128 * oc:128 * (oc + 1)],
                                    rhs=xt[:, d, :],
                                    start=(d == 0),
                                    stop=(d == DCH - 1),
                                )
                            yield ('pe', 253, mm)
                        if oc == 0:
                            yield ('x', 0, lambda t1=t1, ps=ps, ccs=ccs:
                                   nc.vector.tensor_tensor(t1[:], ps, ccs,
                                                           op=MUL))
                            yield ('x', 0, lambda t3=t3, ps=ps, sss=sss:
                                   nc.vector.tensor_tensor(t3[:], ps, sss,
                                                           op=MUL))
                        else:
                            yield ('x', 0, lambda t2=t2, ps=ps, sss=sss:
                                   nc.vector.tensor_tensor(t2[:], ps, sss,
                                                           op=MUL))
                            yield ('x', 0, lambda ro=ro, t1=t1, t2=t2:
                                   nc.vector.tensor_tensor(ro[:, 0, :], t1[:],
                                                           t2[:], op=SUB))
                            yield ('x', 0, lambda t4=t4, ps=ps, ccs=ccs:
                                   nc.vector.tensor_tensor(t4[:], ps, ccs,
                                                           op=MUL))
                            yield ('x', 0, lambda ro=ro, t3=t3, t4=t4:
                                   nc.vector.tensor_tensor(ro[:, 1, :], t3[:],
                                                           t4[:], op=ADD))
                            if t == 0 and wi == 0:
                                # release the dep-gated scalar-ring xt1 load:
                                # the tiny READ of xts[1] makes the DMAs
                                # (emitted right after it) wait via a tracked
                                # WAR dependency
                                def gate_and_load():
                                    nc.vector.tensor_copy(
                                        gate_scr[:, 0:1], xts[1][:, 0:1, 0])
                                    nc.scalar.dma_start(
                                        out=xts[1][:, 0:4, :],
                                        in_=xT_v[:, 0:4, 512:1024])
                                    nc.scalar.dma_start(
                                        out=xts[1][:, 4:8, :],
                                        in_=xT_v[:, 4:8, 512:1024])
                                    nc.scalar.dma_start(
                                        out=wo_t[:],
                                        in_=wo.rearrange("(c p) d -> p c d",
                                                         p=128))
                                    nc.scalar.dma_start(
                                        out=xts[3][:],
                                        in_=xT_v[:, :, 1536:2048])
                                yield ('x', 0, gate_and_load)
                    # repack to per-head-contiguous rows: dst rows
                    # 64*hp + 32*half + j, chunk oc = head pair
                    for half in range(2):
                        for oc in range(2):
                            for hp in range(2):
                                sp = 32 * (2 * oc + hp)
                                dp = 64 * hp + 32 * half
                                def rp_dma(dst=dst, dp=dp, oc=oc, qs=qs,
                                           ro=ro, sp=sp, half=half):
                                    nc.gpsimd.dma_start(
                                        out=dst[dp:dp + 32, oc, qs],
                                        in_=ro[sp:sp + 32, half, :],
                                    )
                                yield ('x', 0, rp_dma)

                for s4 in range(4):
                    s = 4 * t + s4
                    psv = mk_psv(s4)
                    for d in range(DCH):
                        def mmv(psv=psv, d=d, s4=s4):
                            nc.tensor.matmul(
                                psv[:, :GD],
                                lhsT=xt[:, d, 128 * s4:128 * (s4 + 1)],
                                rhs=wv_t[:, d, :],
                                start=(d == 0),
                                stop=(d == DCH - 1),
                            )
                        yield ('pe', 147, mmv)

                    def vpack(psv=psv, s=s):
                        pv = psv[:, :GD].rearrange("p (h e) -> p h e", e=DK)
                        for par, sl in ((0, slice(0, DK)),
                                        (1, slice(DK, 2 * DK))):
                            nc.vector.tensor_copy(
                                v_ext[:, s, par::2, sl], pv[:, par::2, :])
                    yield ('x', 0, vpack)

            # pull machinery: emit 'x' ops freely, 'pe' ops by count quota
            state = {'gen': None, 'peek': None, 'rem': 0}

            def pull_pe(n):
                emitted = 0
                while state['gen'] is not None and emitted < n:
                    if state['peek'] is None:
                        state['peek'] = next(state['gen'], None)
                        if state['peek'] is None:
                            state['gen'] = None
                            break
                    kind, _cost, fn = state['peek']
                    fn()
                    state['peek'] = None
                    if kind == 'pe':
                        emitted += 1
                        state['rem'] -= 1

            # ---------------- A(0) upfront ----------------
            state['gen'] = A_gen(0)
            pull_pe(10 ** 9)
            # remaining v_ext ones columns
            nc.vector.memset(v_ext[:, 4:, 0::2, DK:2 * DK], 1.0)
            nc.vector.memset(v_ext[:, 4:, 1::2, 0:DK], 1.0)

            # ---------------- phases B/C with A(t+1) interleaved ----------
            for t in range(T):
                qs = slice(512 * t, 512 * (t + 1))
                state['gen'] = A_gen(t + 1) if t + 1 < T else None
                state['peek'] = None
                state['rem'] = 64 if t + 1 < T else 0
                slots = 2 * (4 * t + 4) + 4  # weave slots: chunks + C blocks

                def weave():
                    nonlocal slots
                    if slots > 0:
                        if state['rem'] > 0:
                            pull_pe(-(-state['rem'] // slots))
                        slots -= 1

                deferred = []  # previous pair's norm chain (ln/recip/muls)
                for pair in range(2):
                    heads = (2 * pair, 2 * pair + 1)
                    av = avp.tile([128, 2, 512], F32, tag="av",
                                  name=f"av_{t}_{pair}")
                    nck = 4 * t + 4  # eligible kpos chunks
                    pend = []  # [(c, e, f0)] awaiting AV, 4-chunk skew
                    for c in range(nck):
                        if c == 2 and deferred:
                            # emit prev pair's softmax-normalization now: its
                            # ACT ops run behind exp(c0)/exp(c1) and its DVE
                            # muls release the av tile before our first AV.
                            # Pull extra A work to keep PE fed while the ACT
                            # queue chews the 2.3us ln/recip insert.
                            for fn in deferred:
                                fn()
                            deferred = []
                            pull_pe(5)
                        ks = slice(128 * c, 128 * (c + 1))
                        j = c - 4 * t
                        f0 = 128 * j if 0 < j < 4 else 0
                        sc = scp.tile([128, 2, 512], F32, tag="sc")
                        for hp in range(2):
                            rows = slice(64 * hp, 64 * hp + 64)
                            nc.tensor.matmul(
                                sc[:, hp, f0:],
                                lhsT=kT[rows, pair, ks],
                                rhs=qT[rows, pair,
                                       512 * t + f0:512 * (t + 1)],
                                start=True, stop=True,
                                tile_position=(64 * hp, 0),
                            )
                        e = ep.tile([128, 2, 512], MM_DT, tag="e")
                        nc.scalar.activation(e[:, :, f0:], sc[:, :, f0:], EXP)
                        if c >= 4 * t:  # diagonal-crossing tile
                            nc.vector.tensor_tensor(
                                e[:, :, f0:], e[:, :, f0:],
                                msk_t[:, j, :, f0:], op=MUL)
                        pend.append((c, e, f0))
                        if len(pend) > 4:
                            pc, pe_, pf0 = pend.pop(0)
                            for hp in range(2):
                                nc.tensor.matmul(
                                    av[:, hp, pf0:],
                                    lhsT=v_ext[:, pc, heads[hp], :],
                                    rhs=pe_[:, hp, pf0:],
                                    start=(pc == 0),
                                    stop=False,
                                )
                        weave()
                    while pend:
                        pc, pe_, pf0 = pend.pop(0)
                        for hp in range(2):
                            nc.tensor.matmul(
                                av[:, hp, pf0:],
                                lhsT=v_ext[:, pc, heads[hp], :],
                                rhs=pe_[:, hp, pf0:],
                                start=(pc == 0),
                                stop=(not pend),
                            )

                    # softmax normalization: 1/sums = exp(-ln(sums)) over the
                    # WHOLE pair tile (y-rows produce unread NaN garbage).
                    # For pair 1 the yT muls are emitted in 128-col slices so
                    # each phase-C block can start as soon as its slice lands.
                    def norm(av=av, pair=pair, qs=qs, t=t, sliced=(pair == 1)):
                        r1 = rp.tile([128, 2, 512], F32, tag="rr")
                        r2 = rp.tile([128, 2, 512], F32, tag="rr")
                        nc.scalar.activation(r1[:], av[:], LN)
                        nc.scalar.activation(r2[:], r1[:], EXP, scale=-1.0)
                        # head even: y rows 0:64 (slab 0), sums rows 64:128
                        sls = ([(128 * i, 128 * (i + 1)) for i in range(4)]
                               if sliced else [(0, 512)])
                        for lo, hi in sls:
                            ys = slice(512 * t + lo, 512 * t + hi)
                            nc.vector.tensor_tensor(
                                yT[0:64, pair, ys], av[0:64, 0, lo:hi],
                                r2[64:128, 0, lo:hi], op=MUL)
                            nc.vector.tensor_tensor(
                                yT[64:128, pair, ys], av[64:128, 1, lo:hi],
                                r2[0:64, 1, lo:hi], op=MUL)
                    if pair == 0:
                        deferred = [norm]
                    else:
                        norm()
                        pull_pe(6)

                # ---- phase C slice for this qtile ----
                for s_ in range(4 * t, 4 * t + 4):
                    weave()
                    ssl = slice(128 * s_, 128 * (s_ + 1))
                    po = (scp if s_ % 2 == 0 else avp).tile(
                        [128, 2, 512], F32,
                        tag="sc" if s_ % 2 == 0 else "av",
                        name=f"po_{s_}")
                    for n in range(2):
                        nsl = slice(512 * n, 512 * (n + 1))
                        for ldc in range(2):
                            nc.tensor.matmul(
                                po[:, n, :],
                                lhsT=yT[:, ldc, ssl],
                                rhs=wo_t[:, ldc, nsl],
                                start=(ldc == 0),
                                stop=(ldc == 1),
                            )
                        ob = osb.tile([128, 512], F32, tag="ob")
                        if n == 0:
                            nc.vector.tensor_copy(ob[:], po[:, n, :])
                        else:
                            nc.scalar.copy(ob[:], po[:, n, :])
                        nc.sync.dma_start(out=out[ssl, nsl], in_=ob[:])
                # flush any un-absorbed A(t+1) work before the next qtile
                pull_pe(10 ** 9)

    nc.compile()
    return nc


def _host_prep(x, token_positions, Wq, Wk, Wv, Wo):
    # d_k permutation folded into Wq/Wk.  Projection-output row n (0..255):
    # chunk oc = n//128 (all x1 lanes in chunk 0, x2 in chunk 1 for RoPE),
    # head h = (n%128)//32, freq j = n%32 -> orig row 64h + 2j + oc.
    # (The post-RoPE repack DMA then rearranges rows per-head-contiguous.)
    n = np.arange(GD)
    chunk = n // 128
    hh = (n % 128) // 32
    jj = n % 32
    perm = 64 * hh + 2 * jj + chunk

    pos = np.asarray(token_positions).astype(np.float64)
    inv_freq = THETA ** (-np.arange(0, DK, 2, dtype=np.float64) / DK)  # [32]
    ang = pos[:, None] * inv_freq[None, :]                             # [S, 32]
    cos = np.cos(ang).astype(np.float32)
    sin = np.sin(ang).astype(np.float32)
    # [32, S]: rows = freq j (replicated to 128 partitions on device)
    cc = np.ascontiguousarray(cos.T)
    ss = np.ascontiguousarray(sin.T)

    # causal mask tiles for diagonal-crossing chunks: msk[p, j, :, f] = 1.0
    # iff f >= p + 128*j (duplicated across the head-pair dim)
    pp_, ff_ = np.arange(128)[:, None], np.arange(512)[None, :]
    msk1 = np.stack([(ff_ >= pp_ + 128 * j) for j in range(4)], 0)  # [4,128,512]
    msk = np.repeat(msk1.transpose(1, 0, 2)[:, :, None, :], 2, axis=2)

    scale = 1.0 / np.sqrt(np.float32(DK))
    if USE_BF16:
        import ml_dtypes
        mmnp = ml_dtypes.bfloat16
    else:
        mmnp = np.float32
    in_maps = []
    for core in range(8):
        b, g = divmod(core, 4)
        gsl = slice(GD * g, GD * (g + 1))
        in_maps.append({
            "xT": np.ascontiguousarray(np.asarray(x[b], np.float32).T).astype(mmnp),
            "wq": np.ascontiguousarray(
                (np.asarray(Wq[gsl], np.float32) * scale)[perm].T.astype(mmnp)),
            "wk": np.ascontiguousarray(np.asarray(Wk[gsl], np.float32)[perm].T.astype(mmnp)),
            "wv": np.ascontiguousarray(np.asarray(Wv[gsl], np.float32).T.astype(mmnp)),
            "wo": np.ascontiguousarray(np.asarray(Wo[:, gsl], np.float32).T.astype(mmnp)),
            "cc": cc,
            "ss": ss,
            "msk": np.ascontiguousarray(msk).astype(mmnp),
        })
    return in_maps


def kernel(x, token_positions, Wq, Wk, Wv, Wo, _trace=False, _result=[None],
           _tmpdir=None):
    if "nc" not in _cache:
        _cache["nc"] = _build_kernel()
    nc = _cache["nc"]
    in_maps = _host_prep(x, token_positions, Wq, Wk, Wv, Wo)
    res = None
    for attempt in range(3):
        try:
            res = run_bass_kernel_spmd(
                nc, in_maps, core_ids=list(range(8)), trace=_trace,
                tmpdir=_tmpdir)
            break
        except Exception:
            # transient NRT_EXEC_UNIT_UNRECOVERABLE device hiccups resolve
            # on retry
            if attempt == 2:
                raise
    _result[0] = res
    outs = np.stack([r["out"] for r in res.results])  # [8, S, D]
    full = outs.reshape(B, 4, S, D).sum(axis=1, dtype=np.float32)
    return full


# revision 26
# speedup vs baseline: 1.0259x; 1.0259x over previous
"""Causal multihead self-attention with RoPE on 8 TRN2 NeuronCores.

Problem: B=2, S=2048, D=1024, H=16 heads, d_k=64, causal, RoPE theta=10000.

Sharding (Megatron-style): core c = 4*b + g handles batch b and the 4 heads
[4g, 4g+4): Wq/Wk/Wv column-parallel, Wo row-parallel; each core emits a
partial [S, D] output and the host sums the 4 partials per batch.

Device kernel (per core), matmul operands bf16 (fp32 PSUM accumulation):
  A) qT/kT = W' @ x.T in a d-on-partition layout; d_k dims permuted on host
     so RoPE is 6 full-width DVE ops per (tensor, qtile); DMA repack to
     per-head-contiguous rows; v packed as [v | ones] per head (ones columns
     produce softmax sums for free in the AV matmul).
  B) Per (qtile, head pair): scoresT [128 kpos, 2, 512 q] (row-group packed,
     2 heads concurrent on PE), exp on ACT, causal mask via DVE 0/1 multiply,
     AV accumulated in a single [128, 2, 512] PSUM pair-tile. Normalization:
     ONE ln + ONE exp(-x) over the whole pair tile (garbage NaNs in the
     unread y-rows), then 2 DVE muls into yT.
  C) partial = yT.T @ Wo via PE, gpsimd copy to SBUF, DMA out.

Scheduling (what makes this faster than the 197us baseline):
  - All phase-critical loads ride ONE HWDGE ring (sync) in exact need order:
    xt0+wq interleaved first => first matmul starts ~1us in (was 19.7us).
    cos/sin loaded as [32, S] once and replicated on-chip (SBUF->SBUF).
  - 12 dummy warmup matmuls during the initial DMA wait keep/ramp the PE
    HAM clock gate so real matmuls run at 2.4 GHz.
  - Phase B's inner loop is ACT(exp)-bound (~970ns/chunk vs ~640ns PE work),
    so phase A of qtile t+1 is INTERLEAVED into phase B of qtile t via a
    budget-paced generator: projection matmuls fill the PE idle gaps that
    exp would otherwise leave (PE queue is in-order, so emission order is
    the schedule).
  - PSUM plan (8 banks): sc 2x[128,2,512] + av_pair 1x[128,2,512] +
    2x[128,512] for interleaved projections; phase C borrows sc/av slots.

Measured on 8 axon-attached TRN2 cores: see test.py output.
"""
import sys

sys.path.insert(0, "/opt/trn_rl_repo")

import numpy as np

import concourse.bacc as bacc
import concourse.hw_specs as hw_specs
import concourse.tile as tile
from concourse import mybir
from concourse.bass_utils import run_bass_kernel_spmd

# Keep Exp and Ln in one activation-table set: hide them from every other set
# so bacc's table-load pass picks natural_log_exp_and_others for both instead
# of thrashing between exp_and_others and natural_log (~2.7us per reload).
_orig_act_tables = hw_specs.get_activation_tables


def _patched_act_tables(arch):
    _E = mybir.ActivationFunctionType.Exp
    _L = mybir.ActivationFunctionType.Ln
    out = {}
    for name, fns in _orig_act_tables(arch).items():
        if name != "natural_log_exp_and_others":
            fns = fns - {_E, _L}
        out[name] = fns
    return out


bacc.get_activation_tables = _patched_act_tables

F32 = mybir.dt.float32
F32R = mybir.dt.float32r
BF16 = mybir.dt.bfloat16
USE_BF16 = True
MM_DT = BF16 if USE_BF16 else F32R
EXP = mybir.ActivationFunctionType.Exp
LN = mybir.ActivationFunctionType.Ln
MUL = mybir.AluOpType.mult
ADD = mybir.AluOpType.add
SUB = mybir.AluOpType.subtract

B, S, D = 2, 2048, 1024
H, DK = 16, 64          # global heads, head dim
HL = 4                  # heads per core
GD = HL * DK            # local width 256
T = S // 512            # 4 q-tiles of 512
C = S // 128            # 16 kpos chunks of 128
DCH = D // 128          # 8 contraction chunks
THETA = 10000.0

_cache = {}


def _build_kernel():
    nc = bacc.Bacc("TRN2", target_bir_lowering=False, debug=False, num_devices=8)

    xT = nc.declare_dram_parameter("xT", [D, S], MM_DT, isOutput=False)
    wq = nc.declare_dram_parameter("wq", [D, GD], MM_DT, isOutput=False)
    wk = nc.declare_dram_parameter("wk", [D, GD], MM_DT, isOutput=False)
    wv = nc.declare_dram_parameter("wv", [D, GD], MM_DT, isOutput=False)
    wo = nc.declare_dram_parameter("wo", [GD, D], MM_DT, isOutput=False)
    ccd = nc.declare_dram_parameter("cc", [32, S], F32, isOutput=False)
    ssd = nc.declare_dram_parameter("ss", [32, S], F32, isOutput=False)
    mskd = nc.declare_dram_parameter("msk", [128, 4, 2, 512], MM_DT,
                                     isOutput=False)
    out = nc.declare_dram_parameter("out", [S, D], F32, isOutput=True)

    with tile.TileContext(nc) as tc:
        with (
            tc.tile_pool(name="consts", bufs=1) as consts,
            tc.tile_pool(name="persist", bufs=1) as persist,
            tc.tile_pool(name="xtp", bufs=4) as xtp,
            tc.tile_pool(name="rtmp", bufs=8) as rtmp,
            tc.tile_pool(name="rop", bufs=3) as rop,
            tc.tile_pool(name="ep", bufs=8) as ep,
            tc.tile_pool(name="rp", bufs=4) as rp,
            tc.tile_pool(name="osb", bufs=4) as osb,
            tc.tile_pool(name="scp", bufs=2, space="PSUM") as scp,
            tc.tile_pool(name="avp", bufs=1, space="PSUM") as avp,
            tc.tile_pool(name="app", bufs=2, space="PSUM") as app,
        ):
            # ---------------- tiles ----------------
            wq_t = consts.tile([128, DCH, GD], MM_DT, tag="wq")
            wk_t = consts.tile([128, DCH, GD], MM_DT, tag="wk")
            wv_t = consts.tile([128, DCH, GD], MM_DT, tag="wv")
            wo_t = consts.tile([128, 2, D], MM_DT, tag="wo")
            cc_t = consts.tile([128, S], F32, tag="cc")
            ss_t = consts.tile([128, S], F32, tag="ss")
            msk_t = consts.tile([128, 4, 2, 512], MM_DT, tag="msk")
            warm_f = consts.tile([128, 128], F32, tag="warmf")
            warm = consts.tile([128, 128], MM_DT, tag="warm")
            gate_scr = consts.tile([128, 1], MM_DT, tag="gscr")

            qT = persist.tile([128, 2, S], MM_DT, tag="qT")
            kT = persist.tile([128, 2, S], MM_DT, tag="kT")
            yT = persist.tile([128, 2, S], MM_DT, tag="yT")
            v_ext = persist.tile([128, C, HL, 2 * DK], MM_DT, tag="vext")

            xts = [xtp.tile([128, DCH, 512], MM_DT, tag="xt", name=f"xt{t}")
                   for t in range(T)]

            # ---------------- DMA schedule ----------------
            # Three independent DMA rings drain concurrently (round-robin at
            # packet granularity); each ring is strict FIFO, so each carries
            # its loads in exact need order with few, large transfers.
            xT_v = xT.rearrange("(c p) s -> p c s", p=128)
            # sync (HWDGE #1): x tiles 0/2 + wv + msk, then the out stores
            nc.sync.dma_start(out=xts[0][:, 0:2, :], in_=xT_v[:, 0:2, 0:512])
            nc.sync.dma_start(out=xts[0][:, 2:8, :], in_=xT_v[:, 2:8, 0:512])
            nc.sync.dma_start(
                out=wv_t[:], in_=wv.rearrange("(c p) g -> p c g", p=128))
            nc.sync.dma_start(out=msk_t[:], in_=mskd[:])
            nc.sync.dma_start(out=xts[2][:], in_=xT_v[:, :, 1024:1536])
            # scalar (HWDGE #2): weights + x tiles 1/3
            wq_v = wq.rearrange("(c p) g -> p c g", p=128)
            nc.scalar.dma_start(out=wq_t[:, 0:2, :], in_=wq_v[:, 0:2, :])
            nc.scalar.dma_start(out=wq_t[:, 2:8, :], in_=wq_v[:, 2:8, :])
            nc.scalar.dma_start(
                out=wk_t[:], in_=wk.rearrange("(c p) g -> p c g", p=128))
            nc.scalar.dma_start(out=xts[1][:], in_=xT_v[:, :, 512:1024])
            nc.scalar.dma_start(
                out=wo_t[:], in_=wo.rearrange("(c p) d -> p c d", p=128))
            nc.scalar.dma_start(out=xts[3][:], in_=xT_v[:, :, 1536:2048])
            # gpsimd (SWDGE): cos/sin + on-chip replication, then repacks
            nc.gpsimd.dma_start(out=cc_t[0:32, :], in_=ccd[:])
            nc.gpsimd.dma_start(out=ss_t[0:32, :], in_=ssd[:])
            for rb in range(1, 4):
                nc.gpsimd.dma_start(out=cc_t[32 * rb:32 * (rb + 1), :],
                                    in_=cc_t[0:32, :])
                nc.gpsimd.dma_start(out=ss_t[32 * rb:32 * (rb + 1), :],
                                    in_=ss_t[0:32, :])

            # ---------------- PE warmup during initial DMA wait ----------
            nc.vector.memset(warm_f[:], 0.03125)
            nc.vector.tensor_copy(warm[:], warm_f[:])
            wps = app.tile([128, 512], F32, tag="app", name="warmps")
            for i in range(12):
                nc.tensor.matmul(wps[:, 0:128], lhsT=warm[:], rhs=warm[:],
                                 start=(i == 0), stop=(i == 11))

            # v_ext ones halves: even heads cols [64:128], odd heads [0:64].
            # Chunks 0-3 (needed by B(0)) now; the rest after A(0).
            nc.vector.memset(v_ext[:, 0:4, 0::2, DK:2 * DK], 1.0)
            nc.vector.memset(v_ext[:, 0:4, 1::2, 0:DK], 1.0)

            # ---------------- phase A generator ----------------
            # Yields ('pe', est_ns, fn) and ('x', 0, fn) in dependency order.
            def A_gen(t):
                qs = slice(512 * t, 512 * (t + 1))
                xt = xts[t]

                if t == 0:
                    def mk_ps(i):
                        pt = scp.tile([128, 2, 512], F32, tag="sc",
                                      name=f"ps0_{i}")
                        return [pt[:, 0, :], pt[:, 1, :]]

                    pvt = {}

                    def mk_psv(s4):
                        if s4 % 2 == 0:
                            pvt[s4 // 2] = avp.tile([128, 2, 512], F32,
                                                    tag="av",
                                                    name=f"pv0_{s4 // 2}")
                        return pvt[s4 // 2][:, s4 % 2, :]
                else:
                    def mk_ps(i):
                        return [app.tile([128, 512], F32, tag="app",
                                         name=f"ps_{t}_{i}_{oc}")
                                for oc in range(2)]

                    def mk_psv(s4):
                        return app.tile([128, 512], F32, tag="app",
                                        name=f"psv_{t}_{s4}")

                for wi, (w_t, dst) in enumerate(((wq_t, qT), (wk_t, kT))):
                    ccs, sss = cc_t[:, qs], ss_t[:, qs]
                    t1 = rtmp.tile([128, 512], F32, tag="rt")
                    t2 = rtmp.tile([128, 512], F32, tag="rt")
                    t3 = rtmp.tile([128, 512], F32, tag="rt")
                    t4 = rtmp.tile([128, 512], F32, tag="rt")
                    ro = rop.tile([128, 2, 512], MM_DT, tag="ro")
                    pss = mk_ps(wi)
                    for oc in range(2):
                        ps = pss[oc]
                        for d in range(DCH):
                            def mm(ps=ps, w_t=w_t, d=d, oc=oc):
                                nc.tensor.matmul(
                                    ps,
                                    lhsT=w_t[:, d, 128 * oc:128 * (oc + 1)],
                                    rhs=xt[:, d, :],
                                    start=(d == 0),
                                    stop=(d == DCH - 1),
                                )
                            yield ('pe', 253, mm)
                        if oc == 0:
                            yield ('x', 0, lambda t1=t1, ps=ps, ccs=ccs:
                                   nc.vector.tensor_tensor(t1[:], ps, ccs,
                                                           op=MUL))
                            yield ('x', 0, lambda t3=t3, ps=ps, sss=sss:
                                   nc.vector.tensor_tensor(t3[:], ps, sss,
                                                           op=MUL))
                        else:
                            yield ('x', 0, lambda t2=t2, ps=ps, sss=sss:
                                   nc.vector.tensor_tensor(t2[:], ps, sss,
                                                           op=MUL))
                            yield ('x', 0, lambda ro=ro, t1=t1, t2=t2:
                                   nc.vector.tensor_tensor(ro[:, 0, :], t1[:],
                                                           t2[:], op=SUB))
                            yield ('x', 0, lambda t4=t4, ps=ps, ccs=ccs:
                                   nc.vector.tensor_tensor(t4[:], ps, ccs,
                                                           op=MUL))
                            yield ('x', 0, lambda ro=ro, t3=t3, t4=t4:
                                   nc.vector.tensor_tensor(ro[:, 1, :], t3[:],
                                                           t4[:], op=ADD))

                    # repack to per-head-contiguous rows: dst rows
                    # 64*hp + 32*half + j, chunk oc = head pair
                    for half in range(2):
                        for oc in range(2):
                            for hp in range(2):
                                sp = 32 * (2 * oc + hp)
                                dp = 64 * hp + 32 * half
                                def rp_dma(dst=dst, dp=dp, oc=oc, qs=qs,
                                           ro=ro, sp=sp, half=half):
                                    nc.gpsimd.dma_start(
                                        out=dst[dp:dp + 32, oc, qs],
                                        in_=ro[sp:sp + 32, half, :],
                                    )
                                yield ('x', 0, rp_dma)

                for s4 in range(4):
                    s = 4 * t + s4
                    psv = mk_psv(s4)
                    for d in range(DCH):
                        def mmv(psv=psv, d=d, s4=s4):
                            nc.tensor.matmul(
                                psv[:, :GD],
                                lhsT=xt[:, d, 128 * s4:128 * (s4 + 1)],
                                rhs=wv_t[:, d, :],
                                start=(d == 0),
                                stop=(d == DCH - 1),
                            )
                        yield ('pe', 147, mmv)

                    def vpack(psv=psv, s=s):
                        pv = psv[:, :GD].rearrange("p (h e) -> p h e", e=DK)
                        for par, sl in ((0, slice(0, DK)),
                                        (1, slice(DK, 2 * DK))):
                            nc.vector.tensor_copy(
                                v_ext[:, s, par::2, sl], pv[:, par::2, :])
                    yield ('x', 0, vpack)

            # pull machinery: emit 'x' ops freely, 'pe' ops by count quota
            state = {'gen': None, 'peek': None, 'rem': 0}

            def pull_pe(n):
                emitted = 0
                while state['gen'] is not None and emitted < n:
                    if state['peek'] is None:
                        state['peek'] = next(state['gen'], None)
                        if state['peek'] is None:
                            state['gen'] = None
                            break
                    kind, _cost, fn = state['peek']
                    fn()
                    state['peek'] = None
                    if kind == 'pe':
                        emitted += 1
                        state['rem'] -= 1

            # ---------------- A(0) upfront ----------------
            state['gen'] = A_gen(0)
            pull_pe(10 ** 9)
            # remaining v_ext ones columns
            nc.vector.memset(v_ext[:, 4:, 0::2, DK:2 * DK], 1.0)
            nc.vector.memset(v_ext[:, 4:, 1::2, 0:DK], 1.0)

            # ---------------- phases B/C with A(t+1) interleaved ----------
            for t in range(T):
                qs = slice(512 * t, 512 * (t + 1))
                state['gen'] = A_gen(t + 1) if t + 1 < T else None
                state['peek'] = None
                state['rem'] = 64 if t + 1 < T else 0
                slots = 2 * (4 * t + 4) + 4  # weave slots: chunks + C blocks

                def weave():
                    nonlocal slots
                    if slots > 0:
                        if state['rem'] > 0:
                            pull_pe(-(-state['rem'] // slots))
                        slots -= 1

                deferred = []  # previous pair's norm chain (ln/recip/muls)
                for pair in range(2):
                    heads = (2 * pair, 2 * pair + 1)
                    av = avp.tile([128, 2, 512], F32, tag="av",
                                  name=f"av_{t}_{pair}")
                    nck = 4 * t + 4  # eligible kpos chunks
                    pend = []  # [(c, e, f0)] awaiting AV, 4-chunk skew
                    for c in range(nck):
                        if c == 2 and deferred:
                            # emit prev pair's softmax-normalization now: its
                            # ACT ops run behind exp(c0)/exp(c1) and its DVE
                            # muls release the av tile before our first AV.
                            # Pull extra A work to keep PE fed while the ACT
                            # queue chews the 2.3us ln/recip insert.
                            for fn in deferred:
                                fn()
                            deferred = []
                            pull_pe(5)
                        ks = slice(128 * c, 128 * (c + 1))
                        j = c - 4 * t
                        f0 = 128 * j if 0 < j < 4 else 0
                        sc = scp.tile([128, 2, 512], F32, tag="sc")
                        for hp in range(2):
                            rows = slice(64 * hp, 64 * hp + 64)
                            nc.tensor.matmul(
                                sc[:, hp, f0:],
                                lhsT=kT[rows, pair, ks],
                                rhs=qT[rows, pair,
                                       512 * t + f0:512 * (t + 1)],
                                start=True, stop=True,
                                tile_position=(64 * hp, 0),
                            )
                        e = ep.tile([128, 2, 512], MM_DT, tag="e")
                        nc.scalar.activation(e[:, :, f0:], sc[:, :, f0:], EXP)
                        if c >= 4 * t:  # diagonal-crossing tile
                            nc.vector.tensor_tensor(
                                e[:, :, f0:], e[:, :, f0:],
                                msk_t[:, j, :, f0:], op=MUL)
                        pend.append((c, e, f0))
                        if len(pend) > 4:
                            pc, pe_, pf0 = pend.pop(0)
                            for hp in range(2):
                                nc.tensor.matmul(
                                    av[:, hp, pf0:],
                                    lhsT=v_ext[:, pc, heads[hp], :],
                                    rhs=pe_[:, hp, pf0:],
                                    start=(pc == 0),
                                    stop=False,
                                )
                        weave()
                    while pend:
                        pc, pe_, pf0 = pend.pop(0)
                        for hp in range(2):
                            nc.tensor.matmul(
                                av[:, hp, pf0:],
                                lhsT=v_ext[:, pc, heads[hp], :],
                                rhs=pe_[:, hp, pf0:],
                                start=(pc == 0),
                                stop=(not pend),
                            )

                    # softmax normalization: 1/sums = exp(-ln(sums)) over the
                    # WHOLE pair tile (y-rows produce unread NaN garbage).
                    # For pair 1 the yT muls are emitted in 128-col slices so
                    # each phase-C block can start as soon as its slice lands.
                    def norm(av=av, pair=pair, qs=qs, t=t, sliced=(pair == 1)):
                        r1 = rp.tile([128, 2, 512], F32, tag="rr")
                        r2 = rp.tile([128, 2, 512], F32, tag="rr")
                        nc.scalar.activation(r1[:], av[:], LN)
                        nc.scalar.activation(r2[:], r1[:], EXP, scale=-1.0)
                        # head even: y rows 0:64 (slab 0), sums rows 64:128
                        sls = ([(128 * i, 128 * (i + 1)) for i in range(4)]
                               if sliced else [(0, 512)])
                        for lo, hi in sls:
                            ys = slice(512 * t + lo, 512 * t + hi)
                            nc.vector.tensor_tensor(
                                yT[0:64, pair, ys], av[0:64, 0, lo:hi],
                                r2[64:128, 0, lo:hi], op=MUL)
                            nc.vector.tensor_tensor(
                                yT[64:128, pair, ys], av[64:128, 1, lo:hi],
                                r2[0:64, 1, lo:hi], op=MUL)
                    if pair == 0:
                        deferred = [norm]
                    else:
                        norm()
                        pull_pe(6)

                # ---- phase C slice for this qtile ----
                for s_ in range(4 * t, 4 * t + 4):
                    weave()
                    ssl = slice(128 * s_, 128 * (s_ + 1))
                    po = (scp if s_ % 2 == 0 else avp).tile(
                        [128, 2, 512], F32,
                        tag="sc" if s_ % 2 == 0 else "av",
                        name=f"po_{s_}")
                    for n in range(2):
                        nsl = slice(512 * n, 512 * (n + 1))
                        for ldc in range(2):
                            nc.tensor.matmul(
                                po[:, n, :],
                                lhsT=yT[:, ldc, ssl],
                                rhs=wo_t[:, ldc, nsl],
                                start=(ldc == 0),
                                stop=(ldc == 1),
                            )
                        ob = osb.tile([128, 512], F32, tag="ob")
                        if n == 0:
                            nc.vector.tensor_copy(ob[:], po[:, n, :])
                        else:
                            nc.scalar.copy(ob[:], po[:, n, :])
                        nc.sync.dma_start(out=out[ssl, nsl], in_=ob[:])
                # flush any un-absorbed A(t+1) work before the next qtile
                pull_pe(10 ** 9)

    nc.compile()
    return nc


def _host_prep(x, token_positions, Wq, Wk, Wv, Wo):
    # d_k permutation folded into Wq/Wk.  Projection-output row n (0..255):
    # chunk oc = n//128 (all x1 lanes in chunk 0, x2 in chunk 1 for RoPE),
    # head h = (n%128)//32, freq j = n%32 -> orig row 64h + 2j + oc.
    # (The post-RoPE repack DMA then rearranges rows per-head-contiguous.)
    n = np.arange(GD)
    chunk = n // 128
    hh = (n % 128) // 32
    jj = n % 32
    perm = 64 * hh + 2 * jj + chunk

    pos = np.asarray(token_positions).astype(np.float64)
    inv_freq = THETA ** (-np.arange(0, DK, 2, dtype=np.float64) / DK)  # [32]
    ang = pos[:, None] * inv_freq[None, :]                             # [S, 32]
    cos = np.cos(ang).astype(np.float32)
    sin = np.sin(ang).astype(np.float32)
    # [32, S]: rows = freq j (replicated to 128 partitions on device)
    cc = np.ascontiguousarray(cos.T)
    ss = np.ascontiguousarray(sin.T)

    # causal mask tiles for diagonal-crossing chunks: msk[p, j, :, f] = 1.0
    # iff f >= p + 128*j (duplicated across the head-pair dim)
    pp_, ff_ = np.arange(128)[:, None], np.arange(512)[None, :]
    msk1 = np.stack([(ff_ >= pp_ + 128 * j) for j in range(4)], 0)  # [4,128,512]
    msk = np.repeat(msk1.transpose(1, 0, 2)[:, :, None, :], 2, axis=2)

    scale = 1.0 / np.sqrt(np.float32(DK))
    if USE_BF16:
        import ml_dtypes
        mmnp = ml_dtypes.bfloat16
    else:
        mmnp = np.float32
    in_maps = []
    for core in range(8):
        b, g = divmod(core, 4)
        gsl = slice(GD * g, GD * (g + 1))
        in_maps.append({
            "xT": np.ascontiguousarray(np.asarray(x[b], np.float32).T).astype(mmnp),
            "wq": np.ascontiguousarray(
                (np.asarray(Wq[gsl], np.float32) * scale)[perm].T.astype(mmnp)),
            "wk": np.ascontiguousarray(np.asarray(Wk[gsl], np.float32)[perm].T.astype(mmnp)),
            "wv": np.ascontiguousarray(np.asarray(Wv[gsl], np.float32).T.astype(mmnp)),
            "wo": np.ascontiguousarray(np.asarray(Wo[:, gsl], np.float32).T.astype(mmnp)),
            "cc": cc,
            "ss": ss,
            "msk": np.ascontiguousarray(msk).astype(mmnp),
        })
    return in_maps


def kernel(x, token_positions, Wq, Wk, Wv, Wo, _trace=False, _result=[None],
           _tmpdir=None):
    if "nc" not in _cache:
        _cache["nc"] = _build_kernel()
    nc = _cache["nc"]
    in_maps = _host_prep(x, token_positions, Wq, Wk, Wv, Wo)
    res = None
    for attempt in range(3):
        try:
            res = run_bass_kernel_spmd(
                nc, in_maps, core_ids=list(range(8)), trace=_trace,
                tmpdir=_tmpdir)
            break
        except Exception:
            # transient NRT_EXEC_UNIT_UNRECOVERABLE device hiccups resolve
            # on retry
            if attempt == 2:
                raise
    _result[0] = res
    outs = np.stack([r["out"] for r in res.results])  # [8, S, D]
    full = outs.reshape(B, 4, S, D).sum(axis=1, dtype=np.float32)
    return full
